# revision 1
# baseline (speedup 1.0000x reference)
"""Trainium2 Bass kernel for nn_ExperimentalLayer9 (dense transformer layer).

Layer: x + gelu(attn(x) ) @ Wf with
  Q = split_heads(x), K = split_heads(x@Wk+bk), V = split_heads(x@Wv+bv)
  causal softmax (no 1/sqrt(d) scale), exact-erf gelu, residual add.

Sharding over 8 NeuronCores: 2 batch groups x 4-way head/tensor parallel.
Core c handles batch b=c//4 and heads [4r, 4r+4) with r=c%4.  Each core
computes K^T/V projections for its head slice, causal flash-style
attention in transposed-score layout, gelu, and a partial FF over its
1024-row slice of Wf.  A 4-rank ReduceScatter (bf16) sums the FF
partials within each batch group; each core adds the residual x rows for
its rank's 512-row shard and returns that shard.  The host reassembles
the [2, 2048, 1024] output.

All matmuls run in bf16 (fp32 PSUM accumulation); softmax/normalization
in fp32.  exp is computed without max-subtraction (scores are bounded:
std ~5, so exp stays well inside fp32/bf16 range) which avoids any
partition-axis max reduction.  The exp-sum l(q) is obtained for free by
appending a ones-column to V in the attention@V matmul; 1/l is then a
per-partition scalar multiply fused on the vector engine.
"""

import numpy as np
import ml_dtypes

import concourse.bass as bass
import concourse.mybir as mybir
import concourse.tile as tile
from concourse import bacc
from concourse import bass_utils

# Problem shapes (hardcoded per contest contract).
B, S, D, H, DHID = 2, 2048, 1024, 16, 4096
NCORES = 8
GROUP = 4              # cores per batch group
HPC = 4                # heads per core
DK = 64                # q/k head dim
DV = 256               # v head dim
DKS = HPC * DK         # 256  k-slice per core
DVS = HPC * DV         # 1024 v/hidden slice per core
ROWS = S // GROUP      # 512  output rows per core after ReduceScatter
NM = D // 128          # 8    contraction chunks over d_model
VSTRIDE = DV + 1       # 257  V columns per head incl. ones column

BF16 = mybir.dt.bfloat16
F32 = mybir.dt.float32
AF = mybir.ActivationFunctionType

bf16 = ml_dtypes.bfloat16

_compiled = None


def build_program():
    nc = bacc.Bacc(
        "TRN2",
        target_bir_lowering=False,
        debug=False,
        enable_asserts=True,
        num_devices=NCORES,
    )

    # Per-core inputs (values differ per core; program is SPMD-identical).
    xT = nc.dram_tensor("xT", [D, S], BF16, kind="ExternalInput").ap()
    qT = nc.dram_tensor("qT", [DKS, S], BF16, kind="ExternalInput").ap()
    xres = nc.dram_tensor("xres", [ROWS, D], F32, kind="ExternalInput").ap()
    wk = nc.dram_tensor("wk", [D, DKS], BF16, kind="ExternalInput").ap()
    wv = nc.dram_tensor("wv", [D, DVS], BF16, kind="ExternalInput").ap()
    wf = nc.dram_tensor("wf", [DVS, D], BF16, kind="ExternalInput").ap()
    bkb = nc.dram_tensor("bkb", [1, DKS], BF16, kind="ExternalInput").ap()
    bvb = nc.dram_tensor("bvb", [1, DVS], BF16, kind="ExternalInput").ap()
    maskt = nc.dram_tensor("maskt", [128, 128], BF16, kind="ExternalInput").ap()
    ident = nc.dram_tensor("ident", [128, 128], BF16, kind="ExternalInput").ap()
    onesr = nc.dram_tensor("onesr", [1, 512], BF16, kind="ExternalInput").ap()
    out = nc.dram_tensor("out", [ROWS, D], F32, kind="ExternalOutput").ap()

    with tile.TileContext(nc) as tc:
        _body(nc, tc, xT, qT, xres, wk, wv, wf, bkb, bvb, maskt, ident, onesr, out)

    nc.compile()
    return nc


def _body(nc, tc, xT, qT, xres, wk, wv, wf, bkb, bvb, maskt, ident, onesr, out):
    NST = S // 128     # 16 s tiles of 128
    NQT2 = S // 1024   # 2  q tiles of 1024

    with (
        tc.tile_pool(name="const", bufs=1) as constp,
        tc.tile_pool(name="kv", bufs=1) as kvp,
        tc.tile_pool(name="got", bufs=1) as gotp,
        tc.tile_pool(name="res", bufs=1) as resp,
        tc.tile_pool(name="rfp", bufs=2) as rfp,
        tc.tile_pool(name="small", bufs=8) as smallp,
        tc.tile_pool(name="dram", bufs=1, space="DRAM") as dramp,
    ):
        # ---- constants (ACT queue) ------------------------------------
        ones_sb = constp.tile([1, 512], BF16)
        nc.scalar.dma_start(ones_sb[:], onesr[:])
        mask_sb = constp.tile([128, 128], BF16)
        nc.scalar.dma_start(mask_sb[:], maskt[:])
        bk_sb = constp.tile([1, DKS], BF16)
        nc.scalar.dma_start(bk_sb[:], bkb[:])
        bv_sb = constp.tile([1, DVS], BF16)
        nc.scalar.dma_start(bv_sb[:], bvb[:])

        # Warm up the collectives path (ncfw/channel setup) so the first
        # real ReduceScatter doesn't pay ~25us of first-call overhead.
        warm_in = dramp.tile([4, 16], BF16, tag="warm_in")
        warm_out = dramp.tile([1, 16], BF16, tag="warm_out")
        nc.scalar.dma_start(
            warm_in[:].rearrange("a b -> (a b)")[None, :], ones_sb[0:1, 0:64]
        )
        nc.gpsimd.collective_compute(
            "ReduceScatter",
            mybir.AluOpType.add,
            replica_groups=[[0, 1, 2, 3], [4, 5, 6, 7]],
            ins=[warm_in.opt()],
            outs=[warm_out.opt()],
        )

        # [1024, n] DRAM -> [128, 8*n] SBUF, per-chunk DMAs on the Sync
        # queue (all complete before the first xbar transpose issues)
        def load_chunked(pool, src, n):
            t = pool.tile([128, NM * n], src.dtype)
            for m in range(NM):
                nc.sync.dma_start(
                    t[:, m * n : (m + 1) * n],
                    src[m * 128 : (m + 1) * 128, :],
                )
            return t

        qT_sb = kvp.tile([128, 2 * S], BF16)
        for m in range(2):
            nc.sync.dma_start(
                qT_sb[:, m * S : (m + 1) * S], qT[m * 128 : (m + 1) * 128, :]
            )
        kt_sb = kvp.tile([128, 2 * S], BF16)   # K^T rows dk%128, chunk dk//128
        v_sb = kvp.tile([128, NST * HPC * VSTRIDE], BF16)
        got_sb = gotp.tile([128, NM * S], BF16)  # gelu(o)^T, hc-major x q
        # residual x rows: no deps, load early (ACT queue)
        xrs = []
        for g in range(4):
            xr = resp.tile([128, D], F32, tag=f"xr{g}")
            nc.scalar.dma_start(xr[:], xres[g * 128 : (g + 1) * 128, :])
            xrs.append(xr)

        # ---- projections ---------------------------------------------
        with (
            tc.tile_pool(name="projw", bufs=1) as pwp,
            tc.tile_pool(name="xt", bufs=1) as xtp,
            tc.tile_pool(name="psProj", bufs=4, space="PSUM") as psP,
        ):
            wk_sb = load_chunked(pwp, wk, DKS)
            xT_sb = load_chunked(xtp, xT, S)
            wv_sb = load_chunked(pwp, wv, DVS)

            # K^T[dk, s]: lhsT = Wk chunk [128m, 128dk], rhs = xT chunk [128m, 512s]
            for dkt in range(2):
                for st in range(4):
                    ps = psP.tile([128, 512], F32, tag="proj")
                    nc.tensor.matmul(
                        ps[:],
                        bk_sb[:, dkt * 128 : (dkt + 1) * 128],
                        ones_sb[:, 0:512],
                        start=True,
                        stop=False,
                    )
                    for m in range(NM):
                        nc.tensor.matmul(
                            ps[:],
                            wk_sb[:, m * DKS + dkt * 128 : m * DKS + dkt * 128 + 128],
                            xT_sb[:, m * S + st * 512 : m * S + st * 512 + 512],
                            start=False,
                            stop=(m == NM - 1),
                        )
                    nc.scalar.copy(
                        kt_sb[:, dkt * S + st * 512 : dkt * S + st * 512 + 512], ps[:]
                    )

            # V[s, dv] with a ones column per head (col 256 of each strip)
            nc.vector.memset(
                v_sb[:].rearrange("p (t h c) -> p t h c", t=NST, h=HPC)[:, :, :, DV],
                1.0,
            )
            for st in range(NST):
                for dvh in range(2):  # dv halves of 512 = heads (2*dvh, 2*dvh+1)
                    ps = psP.tile([128, 512], F32, tag="proj")
                    nc.tensor.matmul(
                        ps[:],
                        ones_sb[:, 0:128],
                        bv_sb[:, dvh * 512 : dvh * 512 + 512],
                        start=True,
                        stop=False,
                    )
                    for m in range(NM):
                        nc.tensor.matmul(
                            ps[:],
                            xT_sb[:, m * S + st * 128 : m * S + st * 128 + 128],
                            wv_sb[:, m * DVS + dvh * 512 : m * DVS + dvh * 512 + 512],
                            start=False,
                            stop=(m == NM - 1),
                        )
                    base = st * HPC * VSTRIDE
                    for hh in range(2):
                        h = 2 * dvh + hh
                        nc.scalar.copy(
                            v_sb[:, base + h * VSTRIDE : base + h * VSTRIDE + DV],
                            ps[:, hh * 256 : hh * 256 + 256],
                        )

        # ---- attention (head pairs, row-tiled scores) ----------------
        # scores^T[k, q]: contraction is dk=64, so heads 2p (PE rows 0-63)
        # and 2p+1 (rows 64-127) run concurrently via tile_position row
        # tiling.  AV groups run in default 128x128 mode afterwards;
        # exp without max-subtraction; o tiles transposed by xbar DMA.
        with (
            tc.tile_pool(name="expp", bufs=1) as expp,
            tc.tile_pool(name="otile", bufs=4) as otp,
            tc.tile_pool(name="psSt", bufs=3, space="PSUM") as psS,
            tc.tile_pool(name="psAv", bufs=2, space="PSUM") as psV,
        ):
            NQT2 = S // 1024
            for pair in range(2):
                co = pair * S           # both heads of the pair share chunk co

                def st_tile(j, kt, hl, exps):
                    po = 64 * (hl % 2)
                    t = kt - 8 * j   # >=0 on diagonal k-tiles
                    toff = max(t, 0) * 128
                    q0 = j * 1024 + toff
                    ps = psS.tile([128, 1024], F32, tag="st")
                    lo_w = max(0, 512 - toff)
                    if lo_w:
                        nc.tensor.matmul(
                            ps[:, toff : toff + lo_w],
                            kt_sb[po : po + 64, co + kt * 128 : co + kt * 128 + 128],
                            qT_sb[po : po + 64, co + q0 : co + q0 + lo_w],
                            start=True,
                            stop=True,
                            tile_position=(po, 0),
                        )
                    nc.tensor.matmul(
                        ps[:, max(toff, 512) : 1024],
                        kt_sb[po : po + 64, co + kt * 128 : co + kt * 128 + 128],
                        qT_sb[po : po + 64, co + j * 1024 + max(toff, 512) : co + (j + 1) * 1024],
                        start=True,
                        stop=True,
                        tile_position=(po, 0),
                    )
                    nc.scalar.activation(
                        exps[:, kt * 1024 + toff : (kt + 1) * 1024],
                        ps[:, toff:1024],
                        AF.Exp,
                    )
                    if t >= 0:  # mask the diagonal 128x128 block
                        blk = exps[:, kt * 1024 + toff : kt * 1024 + toff + 128]
                        nc.vector.tensor_mul(blk, blk, mask_sb[:])

                def av_tile(j, sq, hl, exps):
                    i = 8 * j + sq
                    pso = psV.tile([128, VSTRIDE], F32, tag="av")
                    for kt in range(i + 1):
                        vb = kt * HPC * VSTRIDE + hl * VSTRIDE
                        nc.tensor.matmul(
                            pso[:],
                            exps[:, kt * 1024 + sq * 128 : kt * 1024 + sq * 128 + 128],
                            v_sb[:, vb : vb + VSTRIDE],
                            start=(kt == 0),
                            stop=(kt == i),
                        )
                    recip = smallp.tile([128, 1], F32, tag="recip")
                    nc.vector.reciprocal(recip[:], pso[:, DV : DV + 1])
                    ot = otp.tile([128, DV], BF16, tag="ot")
                    nc.vector.tensor_scalar_mul(ot[:], pso[:, 0:DV], recip[:])
                    for half in range(2):
                        hc = 2 * hl + half
                        nc.sync.dma_start_transpose(
                            got_sb[:, hc * S + i * 128 : hc * S + i * 128 + 128],
                            ot[:, half * 128 : half * 128 + 128],
                        )

                for j in range(NQT2):   # 1024-wide q tiles
                    hA, hB = 2 * pair, 2 * pair + 1
                    exps_a = expp.tile([128, 16 * 1024], BF16, tag="expSA")
                    exps_b = expp.tile([128, 16 * 1024], BF16, tag="expSB")
                    # row-tiled score phase: both heads stream concurrently
                    for kt in range(8 * j + 8):
                        st_tile(j, kt, hA, exps_a)
                        st_tile(j, kt, hB, exps_b)
                    # default-mode AV phase
                    for sq in range(8):
                        av_tile(j, sq, hA, exps_a)
                        av_tile(j, sq, hB, exps_b)

        # ---- gelu (exact erf) in place on transposed layout ----------
        for hc in range(NM):
            nc.scalar.activation(
                got_sb[:, hc * S : (hc + 1) * S],
                got_sb[:, hc * S : (hc + 1) * S],
                AF.Gelu,
            )

        # ---- FF partial + chunked ReduceScatter + gpsimd residual ----
        with (
            tc.tile_pool(name="ffw", bufs=1) as ffwp,
            tc.tile_pool(name="ffout", bufs=4) as ffoutp,
            tc.tile_pool(name="psFf", bufs=3, space="PSUM") as psF,
        ):
            wf_sb = load_chunked(ffwp, wf, D)
            for g in range(4):
                partial_d = dramp.tile([512, D], BF16, tag=f"part{g}")
                for cc in range(4):
                    c = 4 * g + cc
                    ps0 = psF.tile([128, 512], F32, tag="ff0")
                    ps1 = psF.tile([128, 512], F32, tag="ff1")
                    for hc in range(NM):
                        lhsT = got_sb[:, hc * S + c * 128 : hc * S + c * 128 + 128]
                        nc.tensor.matmul(
                            ps0[:], lhsT, wf_sb[:, hc * D : hc * D + 512],
                            start=(hc == 0), stop=(hc == NM - 1),
                        )
                        nc.tensor.matmul(
                            ps1[:], lhsT, wf_sb[:, hc * D + 512 : hc * D + 1024],
                            start=(hc == 0), stop=(hc == NM - 1),
                        )
                    fo = ffoutp.tile([128, D], BF16, tag="ffout")
                    nc.vector.tensor_copy(fo[:, 0:512], ps0[:])
                    nc.vector.tensor_copy(fo[:, 512:1024], ps1[:])
                    nc.scalar.dma_start(partial_d[cc * 128 : (cc + 1) * 128, :], fo[:])
                rs_d = dramp.tile([128, D], BF16, tag=f"rs{g}")
                nc.gpsimd.collective_compute(
                    "ReduceScatter",
                    mybir.AluOpType.add,
                    replica_groups=[[0, 1, 2, 3], [4, 5, 6, 7]],
                    ins=[partial_d.opt()],
                    outs=[rs_d.opt()],
                )
                # residual: RS-gated cast-DMA on the GpSimd queue (ordered
                # behind this RS), add on DVE, store on ACT
                rf = rfp.tile([128, D], F32, tag="rf")
                nc.gpsimd.dma_start(rf[:], rs_d[:])
                nc.vector.tensor_add(xrs[g][:], xrs[g][:], rf[:])
                nc.scalar.dma_start(out[g * 128 : (g + 1) * 128, :], xrs[g][:])


def make_in_maps(x, Wk, bk, Wv, bv, Wf, bf):
    """Host-side sharding: returns the per-core input dict list."""
    x = np.asarray(x, np.float32)
    Wk = np.asarray(Wk, np.float32)
    Wv = np.asarray(Wv, np.float32)
    Wf = np.asarray(Wf, np.float32)
    bk = np.asarray(bk, np.float32)
    bv = np.asarray(bv, np.float32)
    bf = np.asarray(bf, np.float32)
    mask = np.tril(np.ones((128, 128), np.float32)).T  # mask[k,q]=1 iff k<=q
    in_maps = []
    for c in range(NCORES):
        b, r = c // GROUP, c % GROUP
        xb = x[b]                                    # [S, D]
        xT = np.ascontiguousarray(xb.T).astype(bf16)
        qTs = xT[DKS * r : DKS * (r + 1)]            # heads 4r..4r+3 rows
        # chunked RS: core (b,r) tile g holds x rows 512g+128r+[0,128)
        xres = np.concatenate(
            [xb[512 * g + 128 * r : 512 * g + 128 * r + 128] for g in range(4)]
        ) + bf[None, :].astype(np.float32)
        in_maps.append({
            "xT": xT,
            "qT": np.ascontiguousarray(qTs),
            "xres": np.ascontiguousarray(xres),
            "wk": np.ascontiguousarray(Wk[:, DKS * r : DKS * (r + 1)]).astype(bf16),
            "wv": np.ascontiguousarray(Wv[:, DVS * r : DVS * (r + 1)]).astype(bf16),
            "wf": np.ascontiguousarray(Wf[DVS * r : DVS * (r + 1), :]).astype(bf16),
            "bkb": bk[None, DKS * r : DKS * (r + 1)].astype(bf16),
            "bvb": bv[None, DVS * r : DVS * (r + 1)].astype(bf16),
            "maskt": mask.astype(bf16),
            "ident": np.eye(128, dtype=np.float32).astype(bf16),
            "onesr": np.ones((1, 512), bf16),
        })
    return in_maps


def assemble(results):
    """[8 x [512,1024]] core outputs -> [2,2048,1024]."""
    out = np.empty((B, S, D), np.float32)
    for c in range(NCORES):
        b, r = c // GROUP, c % GROUP
        for g in range(4):
            out[b, 512 * g + 128 * r : 512 * g + 128 * r + 128, :] = results[c][
                "out"
            ][128 * g : 128 * (g + 1)]
    return out


def kernel(x, Wk, bk, Wv, bv, Wf, bf, _trace=False, _trace_cores=None):
    global _compiled
    if _compiled is None:
        _compiled = build_program()
    nc = _compiled
    in_maps = make_in_maps(x, Wk, bk, Wv, bv, Wf, bf)
    res = bass_utils.run_bass_kernel_spmd(
        nc,
        in_maps,
        core_ids=list(range(NCORES)),
        trace=_trace,
        trace_cores=_trace_cores,
    )
    out = assemble(res.results)
    kernel.last_result = res
    return out



# revision 5
# speedup vs baseline: 1.0713x; 1.0713x over previous
"""Trainium2 Bass kernel for nn_ExperimentalLayer9 (dense transformer layer).

Layer: x + gelu(attn(x)) @ Wf with
  Q = split_heads(x), K = split_heads(x@Wk+bk), V = split_heads(x@Wv+bv)
  causal softmax (no 1/sqrt(d) scale), exact-erf gelu, residual add.

Sharding over 8 NeuronCores: 2 batch groups x 4-way head/tensor parallel.
Core c handles batch b=c//4 and heads [4r, 4r+4) with r=c%4.

v2 restructure (HAM/tail-aware):
 - attention@V runs in flipped orientation: V tiles are the stationary
   operand, exp-score strips the moving operand, so the output lands
   directly as o^T[dv, q] in the layout the FF consumes -> no xbar
   transposes, fewer LDWEIGHTS, wide moving strips.
 - softmax denominator l(q) = sum_k exp: per-k-tile running sums on the
   DVE, then one all-ones matmul broadcasts l across partitions; DVE
   reciprocal + multiply normalizes o^T; gelu (with bv folded in as a
   per-partition bias, valid because softmax rows sum to 1) runs on the
   scalar engine right after.
 - FF is chunked into 4 row groups of 512; groups 0/1 are interleaved
   instruction-by-instruction into the j=1 attention phase so their
   ReduceScatters overlap compute; the residual (x+bf)/4 is pre-added to
   every partial before the bf16 RS, so post-RS each core just DMAs its
   shard straight to the output (no post-collective compute tail).
 - output is bf16 (values are bf16-rounded by the RS anyway); host
   upcasts to f32.
"""

import numpy as np
import ml_dtypes

import concourse.bass as bass
import concourse.mybir as mybir
import concourse.tile as tile
from concourse import bacc
from concourse import bass_utils

# Problem shapes (hardcoded per contest contract).
B, S, D, H, DHID = 2, 2048, 1024, 16, 4096
NCORES = 8
GROUP = 4              # cores per batch group
HPC = 4                # heads per core
DK = 64                # q/k head dim
DV = 256               # v head dim
DKS = HPC * DK         # 256  k-slice per core
DVS = HPC * DV         # 1024 v/hidden slice per core
ROWS = S // GROUP      # 512  output rows per core after ReduceScatter
NM = D // 128          # 8    contraction chunks over d_model
NST = S // 128         # 16   s tiles of 128

BF16 = mybir.dt.bfloat16
F32 = mybir.dt.float32
AF = mybir.ActivationFunctionType
RG = [[0, 1, 2, 3], [4, 5, 6, 7]]

bf16 = ml_dtypes.bfloat16

_compiled = None


def build_program():
    nc = bacc.Bacc(
        "TRN2",
        target_bir_lowering=False,
        debug=False,
        enable_asserts=True,
        num_devices=NCORES,
    )

    xT = nc.dram_tensor("xT", [D, S], BF16, kind="ExternalInput").ap()
    qT = nc.dram_tensor("qT", [DKS, S], BF16, kind="ExternalInput").ap()
    xresq = nc.dram_tensor("xresq", [S, D], F32, kind="ExternalInput").ap()
    wk = nc.dram_tensor("wk", [D, DKS], BF16, kind="ExternalInput").ap()
    wv = nc.dram_tensor("wv", [D, DVS], BF16, kind="ExternalInput").ap()
    wf = nc.dram_tensor("wf", [DVS, D], BF16, kind="ExternalInput").ap()
    bkb = nc.dram_tensor("bkb", [1, DKS], BF16, kind="ExternalInput").ap()
    bvp = nc.dram_tensor("bvp", [128, NM], F32, kind="ExternalInput").ap()
    maskt = nc.dram_tensor("maskt", [128, 128], BF16, kind="ExternalInput").ap()
    out = nc.dram_tensor("out", [ROWS, D], BF16, kind="ExternalOutput").ap()

    with tile.TileContext(nc) as tc:
        _body(nc, tc, xT, qT, xresq, wk, wv, wf, bkb, bvp, maskt, out)

    nc.compile()
    return nc


def _body(nc, tc, xT, qT, xresq, wk, wv, wf, bkb, bvp, maskt, out):
    with (
        tc.tile_pool(name="const", bufs=1) as constp,
        tc.tile_pool(name="kv", bufs=1) as kvp,
        tc.tile_pool(name="gotp", bufs=1) as gotp,
        tc.tile_pool(name="wfp", bufs=1) as wfp,
        tc.tile_pool(name="dram", bufs=1, space="DRAM") as dramp,
    ):
        # ---- constants ------------------------------------------------
        ones_sb = constp.tile([1, 512], BF16)
        nc.vector.memset(ones_sb[:], 1.0)
        allones = constp.tile([128, 128], BF16)
        nc.vector.memset(allones[:], 1.0)
        mask_sb = constp.tile([128, 128], BF16)
        nc.scalar.dma_start(mask_sb[:], maskt[:])
        bk_sb = constp.tile([1, DKS], BF16)
        nc.scalar.dma_start(bk_sb[:], bkb[:])
        bv_sb = constp.tile([128, NM], F32)
        nc.scalar.dma_start(bv_sb[:], bvp[:])

        # Warm up the collectives path (ncfw/channel setup) so the first
        # real ReduceScatter doesn't pay ~25us of first-call overhead.
        warm_in = dramp.tile([4, 16], BF16, tag="warm_in")
        warm_out = dramp.tile([1, 16], BF16, tag="warm_out")
        nc.gpsimd.dma_start(
            warm_in[:].rearrange("a b -> (a b)")[None, :], ones_sb[0:1, 0:64]
        )
        nc.gpsimd.collective_compute(
            "ReduceScatter",
            mybir.AluOpType.add,
            replica_groups=RG,
            ins=[warm_in.opt()],
            outs=[warm_out.opt()],
        )

        # ---- persistent SBUF ------------------------------------------
        qT_sb = kvp.tile([128, 2 * S], BF16)
        for m in range(2):
            nc.sync.dma_start(
                qT_sb[:, m * S : (m + 1) * S], qT[m * 128 : (m + 1) * 128, :]
            )
        kt_sb = kvp.tile([128, 2 * S], BF16)   # K^T rows dk%128, chunk dk//128
        # V: col = kt*1024 + h*256 + dv   (per 128-row k tile)
        v_sb = kvp.tile([128, NST * DVS], BF16)
        got_sb = gotp.tile([128, NM * S], BF16)  # gelu(o)^T, hc-major x q
        wf_sb = wfp.tile([128, NM * D], BF16)
        for m in range(NM):  # early load on the gpsimd queue (idle after warmup)
            nc.gpsimd.dma_start(wf_sb[:, m * D : (m + 1) * D], wf[m * 128 : (m + 1) * 128, :])

        # ---- projections ---------------------------------------------
        with (
            tc.tile_pool(name="projw", bufs=1) as pwp,
            tc.tile_pool(name="xtp", bufs=1) as xtp,
            tc.tile_pool(name="psP", bufs=4, space="PSUM") as psP,
        ):
            wk_sb = pwp.tile([128, NM * DKS], BF16)
            for m in range(NM):
                nc.scalar.dma_start(
                    wk_sb[:, m * DKS : (m + 1) * DKS], wk[m * 128 : (m + 1) * 128, :]
                )
            xT_sb = xtp.tile([128, NM * S], BF16)
            for m in range(NM):
                nc.sync.dma_start(
                    xT_sb[:, m * S : (m + 1) * S], xT[m * 128 : (m + 1) * 128, :]
                )
            wv_sb = pwp.tile([128, NM * DVS], BF16)
            for m in range(NM):
                nc.scalar.dma_start(
                    wv_sb[:, m * DVS : (m + 1) * DVS], wv[m * 128 : (m + 1) * 128, :]
                )

            # K^T[dk, s]: lhsT = Wk chunk [128m, 128dk], rhs = xT chunk [128m, 512s]
            for dkt in range(2):
                for st in range(4):
                    ps = psP.tile([128, 512], F32, tag="p")
                    nc.tensor.matmul(
                        ps[:],
                        bk_sb[:, dkt * 128 : (dkt + 1) * 128],
                        ones_sb[:, 0:512],
                        start=True,
                        stop=False,
                    )
                    for m in range(NM):
                        nc.tensor.matmul(
                            ps[:],
                            wk_sb[:, m * DKS + dkt * 128 : m * DKS + dkt * 128 + 128],
                            xT_sb[:, m * S + st * 512 : m * S + st * 512 + 512],
                            start=False,
                            stop=(m == NM - 1),
                        )
                    nc.scalar.copy(
                        kt_sb[:, dkt * S + st * 512 : dkt * S + st * 512 + 512], ps[:]
                    )

            # V[s, dv] without bias (bv is folded into the gelu bias:
            # softmax rows sum to 1, so attn@(xWv + 1*bv) = attn@(xWv) + bv)
            for st in range(NST):
                for dvh in range(2):
                    ps = psP.tile([128, 512], F32, tag="p")
                    for m in range(NM):
                        nc.tensor.matmul(
                            ps[:],
                            xT_sb[:, m * S + st * 128 : m * S + st * 128 + 128],
                            wv_sb[:, m * DVS + dvh * 512 : m * DVS + dvh * 512 + 512],
                            start=(m == 0),
                            stop=(m == NM - 1),
                        )
                    nc.scalar.copy(
                        v_sb[:, st * DVS + dvh * 512 : st * DVS + dvh * 512 + 512],
                        ps[:],
                    )

        # ---- attention + FF (interleaved) ----------------------------
        with (
            tc.tile_pool(name="expp", bufs=1) as expp,
            tc.tile_pool(name="accp", bufs=1) as accp,
            tc.tile_pool(name="rcpp", bufs=1) as rcpp,
            tc.tile_pool(name="xrp", bufs=4) as xrp,
            tc.tile_pool(name="fop", bufs=2) as fop,
            tc.tile_pool(name="psS", bufs=2, space="PSUM") as psS,
            tc.tile_pool(name="psV", bufs=1, space="PSUM") as psV,
            tc.tile_pool(name="psF", bufs=2, space="PSUM") as psF,
        ):
            partials = [
                dramp.tile([512, D], BF16, tag=f"part{g}", name=f"part{g}")
                for g in range(4)
            ]
            rss = [
                dramp.tile([128, D], BF16, tag=f"rs{g}", name=f"rs{g}")
                for g in range(4)
            ]

            def alloc_pair():
                exps2, acc2 = [], []
                for hl in range(2):
                    e = expp.tile([128, NST * 1024], BF16, tag=f"e{hl}")
                    a = accp.tile([128, 1024], F32, tag=f"a{hl}")
                    exps2.append(e)
                    acc2.append(a)
                return exps2, acc2

            def emit_scores_pair(pair, j, exps2, acc2, filler=None):
                """Causal scores^T -> exp (no max-sub) -> mask -> running
                k-sums.  Two heads run row-tiled on the 64-row PE halves."""
                nkt = 8 * j + 8
                co = pair * S
                for kt in range(nkt):
                    t = kt - 8 * j
                    toff = max(t, 0) * 128
                    for hl in range(2):
                        po = 64 * hl
                        for qh in range(2):
                            a = max(qh * 512, toff)
                            b_ = qh * 512 + 512
                            if a >= b_:
                                continue
                            ps = psS.tile([128, 512], F32, tag="st")
                            nc.tensor.matmul(
                                ps[:, a - qh * 512 : 512],
                                kt_sb[po : po + 64, co + kt * 128 : co + kt * 128 + 128],
                                qT_sb[po : po + 64, co + j * 1024 + a : co + j * 1024 + b_],
                                start=True,
                                stop=True,
                                tile_position=(po, 0),
                            )
                            nc.scalar.activation(
                                exps2[hl][:, kt * 1024 + a : kt * 1024 + b_],
                                ps[:, a - qh * 512 : 512],
                                AF.Exp,
                            )
                        if t >= 0:  # mask the diagonal 128x128 block
                            blk = exps2[hl][:, kt * 1024 + toff : kt * 1024 + toff + 128]
                            nc.vector.tensor_mul(blk, blk, mask_sb[:])
                        # running sum over k tiles (l accumulators)
                        if kt == 0:
                            nc.vector.tensor_copy(acc2[hl][:], exps2[hl][:, 0:1024])
                        else:
                            nc.vector.tensor_add(
                                acc2[hl][:, toff:1024],
                                acc2[hl][:, toff:1024],
                                exps2[hl][:, kt * 1024 + toff : (kt + 1) * 1024],
                            )
                        if filler is not None:
                            next(filler, None)

            def emit_head(h, j, exps, acc):
                """Flipped AV (V stationary) + l broadcast + normalize."""
                pair, hl = divmod(h, 2)
                nkt = 8 * j + 8
                psvs = []
                for c in range(2):
                    psv = psV.tile([128, 1024], F32, tag=f"av{c}")
                    for kt in range(nkt):
                        toff = max(kt - 8 * j, 0) * 128
                        vcol = kt * DVS + h * 256 + c * 128
                        for qh in range(2):
                            a = max(qh * 512, toff)
                            b_ = qh * 512 + 512
                            if a >= b_:
                                continue
                            nc.tensor.matmul(
                                psv[:, a:b_],
                                v_sb[:, vcol : vcol + 128],
                                exps[:, kt * 1024 + a : kt * 1024 + b_],
                                start=(kt == 0),
                                stop=(kt == nkt - 1),
                                skip_group_check=True,
                            )
                    psvs.append(psv)
                # l(q) replicated across partitions: allones.T @ acc
                accb = accp.tile([128, 1024], BF16, tag=f"ab{hl}")
                nc.scalar.copy(accb[:], acc[:])
                rcp = rcpp.tile([128, 1024], F32, tag=f"r{hl}")
                for half in range(2):
                    rb = psS.tile([128, 512], F32, tag="st")
                    nc.tensor.matmul(
                        rb[:], allones[:], accb[:, half * 512 : half * 512 + 512],
                        start=True, stop=True,
                    )
                    nc.vector.reciprocal(rcp[:, half * 512 : half * 512 + 512], rb[:])
                # normalize into got (bf16); gelu is emitted separately
                for c in range(2):
                    gs = got_sb[:, (2 * h + c) * S + j * 1024 : (2 * h + c) * S + j * 1024 + 1024]
                    nc.vector.tensor_mul(gs, psvs[c][:], rcp[:])

            def emit_gelu(h, j):
                for c in range(2):
                    hc = 2 * h + c
                    gs = got_sb[:, hc * S + j * 1024 : hc * S + j * 1024 + 1024]
                    nc.scalar.activation(gs, gs, AF.Gelu, bias=bv_sb[:, hc : hc + 1])

            def ff_gen(g):
                """FF partial for row group g, ~2 matmuls per pull."""
                for cc in range(4):
                    q0 = g * 512 + cc * 128
                    xr = xrp.tile([128, D], F32, tag="xr")
                    nc.scalar.dma_start(xr[:], xresq[q0 : q0 + 128, :])
                    ps0 = psF.tile([128, 512], F32, tag="ff")
                    ps1 = psF.tile([128, 512], F32, tag="ff")
                    for hc in range(NM):
                        lhsT = got_sb[:, hc * S + q0 : hc * S + q0 + 128]
                        nc.tensor.matmul(
                            ps0[:], lhsT, wf_sb[:, hc * D : hc * D + 512],
                            start=(hc == 0), stop=(hc == NM - 1),
                        )
                        nc.tensor.matmul(
                            ps1[:], lhsT, wf_sb[:, hc * D + 512 : hc * D + 1024],
                            start=(hc == 0), stop=(hc == NM - 1),
                        )
                        yield
                    # pre-add (x+bf)/4 so the RS carries the residual
                    fo = fop.tile([128, D], BF16, tag="fo")
                    nc.vector.tensor_add(fo[:, 0:512], ps0[:], xr[:, 0:512])
                    nc.vector.tensor_add(fo[:, 512:1024], ps1[:], xr[:, 512:1024])
                    nc.sync.dma_start(partials[g][cc * 128 : (cc + 1) * 128, :], fo[:])

            def emit_rs(g):
                nc.gpsimd.collective_compute(
                    "ReduceScatter",
                    mybir.AluOpType.add,
                    replica_groups=RG,
                    ins=[partials[g].opt()],
                    outs=[rss[g].opt()],
                )
                nc.gpsimd.dma_start(out[g * 128 : (g + 1) * 128, :], rss[g][:])

            # ---- j = 0 (q rows 0..1023) ----
            e0, a0 = alloc_pair()
            emit_scores_pair(0, 0, e0, a0)
            emit_head(0, 0, e0[0], a0[0])
            emit_head(1, 0, e0[1], a0[1])
            e1, a1 = alloc_pair()
            emit_scores_pair(1, 0, e1, a1)
            emit_gelu(0, 0)
            emit_gelu(1, 0)
            emit_head(2, 0, e1[0], a1[0])
            emit_head(3, 0, e1[1], a1[1])
            emit_gelu(2, 0)
            emit_gelu(3, 0)

            # ---- j = 1 (q rows 1024..2047), FF g0/g1 interleaved ----
            e0, a0 = alloc_pair()
            f0 = ff_gen(0)
            emit_scores_pair(0, 1, e0, a0, filler=f0)
            for _ in f0:
                pass
            emit_rs(0)
            emit_head(0, 1, e0[0], a0[0])
            emit_head(1, 1, e0[1], a0[1])
            e1, a1 = alloc_pair()
            f1 = ff_gen(1)
            emit_scores_pair(1, 1, e1, a1, filler=f1)
            for _ in f1:
                pass
            emit_rs(1)
            emit_gelu(0, 1)
            emit_gelu(1, 1)
            emit_head(2, 1, e1[0], a1[0])
            emit_head(3, 1, e1[1], a1[1])
            emit_gelu(2, 1)
            emit_gelu(3, 1)

            # ---- FF tail: groups 2 and 3 ----
            for _ in ff_gen(2):
                pass
            emit_rs(2)
            for _ in ff_gen(3):
                pass
            emit_rs(3)


def make_in_maps(x, Wk, bk, Wv, bv, Wf, bf):
    """Host-side sharding: returns the per-core input dict list."""
    x = np.asarray(x, np.float32)
    Wk = np.asarray(Wk, np.float32)
    Wv = np.asarray(Wv, np.float32)
    Wf = np.asarray(Wf, np.float32)
    bk = np.asarray(bk, np.float32)
    bv = np.asarray(bv, np.float32)
    bf = np.asarray(bf, np.float32)
    mask = np.tril(np.ones((128, 128), np.float32)).T  # mask[k,q]=1 iff k<=q
    in_maps = []
    for c in range(NCORES):
        b, r = c // GROUP, c % GROUP
        xb = x[b]                                    # [S, D]
        xT = np.ascontiguousarray(xb.T).astype(bf16)
        qTs = xT[DKS * r : DKS * (r + 1)]            # heads 4r..4r+3 rows
        bv_s = bv[DVS * r : DVS * (r + 1)]
        in_maps.append({
            "xT": xT,
            "qT": np.ascontiguousarray(qTs),
            "xresq": np.ascontiguousarray((xb + bf[None, :]) * 0.25),
            "wk": np.ascontiguousarray(Wk[:, DKS * r : DKS * (r + 1)]).astype(bf16),
            "wv": np.ascontiguousarray(Wv[:, DVS * r : DVS * (r + 1)]).astype(bf16),
            "wf": np.ascontiguousarray(Wf[DVS * r : DVS * (r + 1), :]).astype(bf16),
            "bkb": bk[None, DKS * r : DKS * (r + 1)].astype(bf16),
            "bvp": np.ascontiguousarray(bv_s.reshape(NM, 128).T).astype(np.float32),
            "maskt": mask.astype(bf16),
        })
    return in_maps


def assemble(results):
    """[8 x [512,1024] bf16] core outputs -> [2,2048,1024] f32."""
    out = np.empty((B, S, D), np.float32)
    for c in range(NCORES):
        b, r = c // GROUP, c % GROUP
        res = np.asarray(results[c]["out"], dtype=np.float32)
        for g in range(4):
            out[b, 512 * g + 128 * r : 512 * g + 128 * r + 128, :] = res[
                128 * g : 128 * (g + 1)
            ]
    return out


def kernel(x, Wk, bk, Wv, bv, Wf, bf, _trace=False, _trace_cores=None):
    global _compiled
    if _compiled is None:
        _compiled = build_program()
    nc = _compiled
    in_maps = make_in_maps(x, Wk, bk, Wv, bv, Wf, bf)
    res = bass_utils.run_bass_kernel_spmd(
        nc,
        in_maps,
        core_ids=list(range(NCORES)),
        trace=_trace,
        trace_cores=_trace_cores,
    )
    out = assemble(res.results)
    kernel.last_result = res
    return out


# revision 7
# speedup vs baseline: 1.1905x; 1.1113x over previous
"""Trainium2 Bass kernel for nn_ExperimentalLayer9 (dense transformer layer).

Layer: x + gelu(attn(x)) @ Wf with
  Q = split_heads(x), K = split_heads(x@Wk+bk), V = split_heads(x@Wv+bv)
  causal softmax (no 1/sqrt(d) scale), exact-erf gelu, residual add.

Sharding over 8 NeuronCores: 2 batch groups x 4-way head/tensor parallel.
Core c handles batch b=c//4 and heads [4r, 4r+4) with r=c%4.

v3: single shared PSUM scope + instruction-level interleaving so the PE
never waits on the scalar-engine exp stream:
 - attention@V in flipped orientation (V stationary, exp-score strips
   moving) -> o^T lands pre-transposed for the FF, no xbar transposes.
 - V projection tiles are pulled as PE fillers inside the j=0 score
   loops (their exp runs concurrently on the scalar engine); FF row
   groups 0/1 are pulled as fillers inside the j=1 attention phase so
   both ReduceScatters overlap compute.
 - softmax denominator: bf16 running k-sums on the DVE, one all-ones
   matmul broadcasts l across partitions, reciprocal_approx_fast (5x
   cheaper than reciprocal), multiply+gelu(+bv as per-partition bias).
 - residual (x+bf)/4 pre-added to every FF partial before the bf16 RS:
   post-RS each core DMAs its shard straight to the bf16 output.
"""

import numpy as np
import ml_dtypes

import concourse.bass as bass
import concourse.mybir as mybir
import concourse.tile as tile
from concourse import bacc
from concourse import bass_utils

# Problem shapes (hardcoded per contest contract).
B, S, D, H, DHID = 2, 2048, 1024, 16, 4096
NCORES = 8
GROUP = 4              # cores per batch group
HPC = 4                # heads per core
DK = 64                # q/k head dim
DV = 256               # v head dim
DKS = HPC * DK         # 256  k-slice per core
DVS = HPC * DV         # 1024 v/hidden slice per core
ROWS = S // GROUP      # 512  output rows per core after ReduceScatter
NM = D // 128          # 8    contraction chunks over d_model
NST = S // 128         # 16   s tiles of 128

BF16 = mybir.dt.bfloat16
F32 = mybir.dt.float32
AF = mybir.ActivationFunctionType
RG = [[0, 1, 2, 3], [4, 5, 6, 7]]

bf16 = ml_dtypes.bfloat16

_compiled = None


def build_program():
    nc = bacc.Bacc(
        "TRN2",
        target_bir_lowering=False,
        debug=False,
        enable_asserts=True,
        num_devices=NCORES,
    )

    xT = nc.dram_tensor("xT", [D, S], BF16, kind="ExternalInput").ap()
    qT = nc.dram_tensor("qT", [DKS, S], BF16, kind="ExternalInput").ap()
    xresq = nc.dram_tensor("xresq", [S, D], F32, kind="ExternalInput").ap()
    wk = nc.dram_tensor("wk", [D, DKS], BF16, kind="ExternalInput").ap()
    wv = nc.dram_tensor("wv", [D, DVS], BF16, kind="ExternalInput").ap()
    wf = nc.dram_tensor("wf", [DVS, D], BF16, kind="ExternalInput").ap()
    bkb = nc.dram_tensor("bkb", [1, DKS], BF16, kind="ExternalInput").ap()
    bvp = nc.dram_tensor("bvp", [128, NM], F32, kind="ExternalInput").ap()
    maskt = nc.dram_tensor("maskt", [128, 128], BF16, kind="ExternalInput").ap()
    out = nc.dram_tensor("out", [ROWS, D], BF16, kind="ExternalOutput").ap()

    with tile.TileContext(nc) as tc:
        _body(nc, tc, xT, qT, xresq, wk, wv, wf, bkb, bvp, maskt, out)

    nc.compile()
    return nc


def _body(nc, tc, xT, qT, xresq, wk, wv, wf, bkb, bvp, maskt, out):
    with (
        tc.tile_pool(name="const", bufs=1) as constp,
        tc.tile_pool(name="kv", bufs=1) as kvp,
        tc.tile_pool(name="gotp", bufs=1) as gotp,
        tc.tile_pool(name="wfp", bufs=1) as wfp,
        tc.tile_pool(name="accp", bufs=2) as accp,
        tc.tile_pool(name="rcpp", bufs=1) as rcpp,
        tc.tile_pool(name="dram", bufs=1, space="DRAM") as dramp,
        tc.tile_pool(name="psS", bufs=2, space="PSUM") as psS,
        tc.tile_pool(name="psV", bufs=1, space="PSUM") as psV,
        tc.tile_pool(name="psF", bufs=2, space="PSUM") as psF,
    ):
        # ---- constants ------------------------------------------------
        ones_sb = constp.tile([1, 512], BF16)
        nc.vector.memset(ones_sb[:], 1.0)
        allones = constp.tile([128, 128], BF16)
        nc.vector.memset(allones[:], 1.0)
        mask_sb = constp.tile([128, 128], BF16)
        nc.scalar.dma_start(mask_sb[:], maskt[:])
        bk_sb = constp.tile([1, DKS], BF16)
        nc.scalar.dma_start(bk_sb[:], bkb[:])
        bv_sb = constp.tile([128, NM], F32)
        nc.scalar.dma_start(bv_sb[:], bvp[:])

        # ---- persistent SBUF + loads ----------------------------------
        qT_sb = kvp.tile([128, 2 * S], BF16)
        kt_sb = kvp.tile([128, 2 * S], BF16)   # K^T rows dk%128, chunk dk//128
        # V: col = kt*1024 + h*256 + dv   (per 128-row k tile)
        v_sb = kvp.tile([128, NST * DVS], BF16)
        got_sb = gotp.tile([128, NM * S], BF16)  # gelu(o)^T, hc-major x q
        wf_sb = wfp.tile([128, NM * D], BF16)

        # gpsimd queue: wk (needed first), then collective warmup, then wf
        wk_sb = kvp.tile([128, NM * DKS], BF16)
        for m in range(NM):
            nc.gpsimd.dma_start(
                wk_sb[:, m * DKS : (m + 1) * DKS], wk[m * 128 : (m + 1) * 128, :]
            )
        warm_in = dramp.tile([4, 16], BF16, tag="warm_in")
        warm_out = dramp.tile([1, 16], BF16, tag="warm_out")
        nc.gpsimd.dma_start(
            warm_in[:].rearrange("a b -> (a b)")[None, :], ones_sb[0:1, 0:64]
        )
        nc.gpsimd.collective_compute(
            "ReduceScatter",
            mybir.AluOpType.add,
            replica_groups=RG,
            ins=[warm_in.opt()],
            outs=[warm_out.opt()],
        )
        for m in range(NM):
            nc.gpsimd.dma_start(
                wf_sb[:, m * D : (m + 1) * D], wf[m * 128 : (m + 1) * 128, :]
            )

        partials = [
            dramp.tile([512, D], BF16, tag=f"part{g}", name=f"part{g}")
            for g in range(4)
        ]
        rss = [
            dramp.tile([128, D], BF16, tag=f"rs{g}", name=f"rs{g}")
            for g in range(4)
        ]

        # ---- emission helpers -----------------------------------------
        def emit_scores_pair(pair, j, exps2, acc2, filler=None):
            """Causal scores^T -> exp (no max-sub) -> mask -> bf16 running
            k-sums.  Two heads run row-tiled on the 64-row PE halves.
            One filler unit is pulled per (kt, hl) to keep the PE fed
            while the scalar engine streams exp."""
            nkt = 8 * j + 8
            co = pair * S
            for kt in range(nkt):
                t = kt - 8 * j
                toff = max(t, 0) * 128
                for hl in range(2):
                    po = 64 * hl
                    for qh in range(2):
                        a = max(qh * 512, toff)
                        b_ = qh * 512 + 512
                        if a >= b_:
                            continue
                        ps = psS.tile([128, 512], F32, tag="st", name="st")
                        nc.tensor.matmul(
                            ps[:, a - qh * 512 : 512],
                            kt_sb[po : po + 64, co + kt * 128 : co + kt * 128 + 128],
                            qT_sb[po : po + 64, co + j * 1024 + a : co + j * 1024 + b_],
                            start=True,
                            stop=True,
                            tile_position=(po, 0),
                        )
                        nc.scalar.activation(
                            exps2[hl][:, kt * 1024 + a : kt * 1024 + b_],
                            ps[:, a - qh * 512 : 512],
                            AF.Exp,
                        )
                    if t >= 0:  # mask the diagonal 128x128 block
                        blk = exps2[hl][:, kt * 1024 + toff : kt * 1024 + toff + 128]
                        nc.vector.tensor_mul(blk, blk, mask_sb[:])
                    # bf16 running sum over k tiles (softmax denominator)
                    if kt == 0:
                        nc.vector.tensor_copy(acc2[hl][:], exps2[hl][:, 0:1024])
                    else:
                        nc.vector.tensor_add(
                            acc2[hl][:, toff:1024],
                            acc2[hl][:, toff:1024],
                            exps2[hl][:, kt * 1024 + toff : (kt + 1) * 1024],
                        )
                    if filler is not None:
                        next(filler, None)

        def emit_head(h, j, exps, acc, filler=None):
            """Flipped AV (V stationary) + l broadcast + normalize.
            l-matmul + fast reciprocal are emitted between the two dv
            chunks so the reciprocal overlaps chunk-1 matmuls and the
            psV slot for the next head frees immediately."""
            nkt = 8 * j + 8
            psvs = []
            rcp = rcpp.tile([128, 1024], F32, tag=f"r{h % 2}", name="rcp")
            for c in range(2):
                psv = psV.tile([128, 1024], F32, tag=f"av{c}", name="psv")
                for kt in range(nkt):
                    toff = max(kt - 8 * j, 0) * 128
                    vcol = kt * DVS + h * 256 + c * 128
                    for qh in range(2):
                        a = max(qh * 512, toff)
                        b_ = qh * 512 + 512
                        if a >= b_:
                            continue
                        nc.tensor.matmul(
                            psv[:, a:b_],
                            v_sb[:, vcol : vcol + 128],
                            exps[:, kt * 1024 + a : kt * 1024 + b_],
                            start=(kt == 0),
                            stop=(kt == nkt - 1),
                            skip_group_check=True,
                        )
                    if c == 0 and filler is not None:
                        next(filler, None)
                psvs.append(psv)
                if c == 0:
                    # l(q) replicated across partitions: allones.T @ acc
                    for half in range(2):
                        rb = psS.tile([128, 512], F32, tag="st", name="rb")
                        nc.tensor.matmul(
                            rb[:], allones[:], acc[:, half * 512 : half * 512 + 512],
                            start=True, stop=True,
                        )
                        nc.vector.reciprocal_approx_fast(
                            rcp[:, half * 512 : half * 512 + 512], rb[:]
                        )
            for c in range(2):
                gs = got_sb[
                    :, (2 * h + c) * S + j * 1024 : (2 * h + c) * S + j * 1024 + 1024
                ]
                nc.vector.tensor_mul(gs, psvs[c][:], rcp[:])

        def emit_gelu(j):
            for hc in range(NM):
                gs = got_sb[:, hc * S + j * 1024 : hc * S + j * 1024 + 1024]
                nc.scalar.activation(gs, gs, AF.Gelu, bias=bv_sb[:, hc : hc + 1])

        def ff_gen(g, xrp, fop):
            """FF partial for row group g; one yield per 2 matmuls."""
            for cc in range(4):
                q0 = g * 512 + cc * 128
                xr = xrp.tile([128, D], F32, tag="xr", name="xr")
                nc.sync.dma_start(xr[:], xresq[q0 : q0 + 128, :])
                ps0 = psF.tile([128, 512], F32, tag="ff", name="ps0")
                ps1 = psF.tile([128, 512], F32, tag="ff", name="ps1")
                for hc in range(NM):
                    lhsT = got_sb[:, hc * S + q0 : hc * S + q0 + 128]
                    nc.tensor.matmul(
                        ps0[:], lhsT, wf_sb[:, hc * D : hc * D + 512],
                        start=(hc == 0), stop=(hc == NM - 1),
                    )
                    nc.tensor.matmul(
                        ps1[:], lhsT, wf_sb[:, hc * D + 512 : hc * D + 1024],
                        start=(hc == 0), stop=(hc == NM - 1),
                    )
                    yield
                # pre-add (x+bf)/4 so the RS carries the residual
                fo = fop.tile([128, D], BF16, tag="fo", name="fo")
                nc.vector.tensor_add(fo[:, 0:512], ps0[:], xr[:, 0:512])
                nc.vector.tensor_add(fo[:, 512:1024], ps1[:], xr[:, 512:1024])
                nc.sync.dma_start(partials[g][cc * 128 : (cc + 1) * 128, :], fo[:])

        def emit_rs(g):
            nc.gpsimd.collective_compute(
                "ReduceScatter",
                mybir.AluOpType.add,
                replica_groups=RG,
                ins=[partials[g].opt()],
                outs=[rss[g].opt()],
            )
            nc.gpsimd.dma_start(out[g * 128 : (g + 1) * 128, :], rss[g][:])

        # ================ phase A: projections + j = 0 ================
        with (
            tc.tile_pool(name="xtp", bufs=1) as xtp,
            tc.tile_pool(name="expj0", bufs=1) as expj0,
        ):
            xT_sb = xtp.tile([128, NM * S], BF16)
            for m in range(NM):  # sync queue: xT first (K proj pacing)
                nc.sync.dma_start(
                    xT_sb[:, m * S : (m + 1) * S], xT[m * 128 : (m + 1) * 128, :]
                )
            for m in range(2):
                nc.sync.dma_start(
                    qT_sb[:, m * S : (m + 1) * S], qT[m * 128 : (m + 1) * 128, :]
                )
            wv_sb = xtp.tile([128, NM * DVS], BF16)
            for m in range(NM):
                nc.scalar.dma_start(
                    wv_sb[:, m * DVS : (m + 1) * DVS], wv[m * 128 : (m + 1) * 128, :]
                )

            # K^T proj: 2 passes of 4 st-tiles, m-outer so the PE starts
            # as soon as xT chunk 0 lands (bias seeds run immediately).
            for dkt in range(2):
                ktile = []
                for st in range(4):
                    pool = psS if st < 2 else psF
                    kps = pool.tile([128, 512], F32, tag="st" if st < 2 else "ff", name="kps")
                    nc.tensor.matmul(
                        kps[:],
                        bk_sb[:, dkt * 128 : (dkt + 1) * 128],
                        ones_sb[:, 0:512],
                        start=True,
                        stop=False,
                    )
                    ktile.append(kps)
                for m in range(NM):
                    for st in range(4):
                        nc.tensor.matmul(
                            ktile[st][:],
                            wk_sb[:, m * DKS + dkt * 128 : m * DKS + dkt * 128 + 128],
                            xT_sb[:, m * S + st * 512 : m * S + st * 512 + 512],
                            start=False,
                            stop=(m == NM - 1),
                        )
                for st in range(4):
                    nc.vector.tensor_copy(
                        kt_sb[:, dkt * S + st * 512 : dkt * S + st * 512 + 512],
                        ktile[st][:],
                    )

            def v_gen(st0, st1):
                """V proj units (st, dvh): 8 matmuls + a DVE copy each."""
                for st in range(st0, st1):
                    for dvh in range(2):
                        vps = psV.tile(
                            [128, 512], F32, tag=f"av{dvh}", name="vps"
                        )
                        for m in range(NM):
                            nc.tensor.matmul(
                                vps[:],
                                xT_sb[:, m * S + st * 128 : m * S + st * 128 + 128],
                                wv_sb[:, m * DVS + dvh * 512 : m * DVS + dvh * 512 + 512],
                                start=(m == 0),
                                stop=(m == NM - 1),
                            )
                        nc.vector.tensor_copy(
                            v_sb[:, st * DVS + dvh * 512 : st * DVS + dvh * 512 + 512],
                            vps[:],
                        )
                        yield

            e_j0 = [
                expj0.tile([128, 8 * 1024], BF16, tag=f"ej0{hl}", name="ej0")
                for hl in range(2)
            ]
            a_j0 = [
                accp.tile([128, 1024], BF16, tag=f"a{hl}", name="accj0")
                for hl in range(2)
            ]
            # pair 0 scores with V st0-7 as fillers (16 pulls, 16 units)
            vg = v_gen(0, 8)
            emit_scores_pair(0, 0, e_j0, a_j0, filler=vg)
            for _ in vg:
                pass
            emit_head(0, 0, e_j0[0], a_j0[0])
            emit_head(1, 0, e_j0[1], a_j0[1])
            e_j0b = [
                expj0.tile([128, 8 * 1024], BF16, tag=f"ej0{hl}", name="ej0b")
                for hl in range(2)
            ]
            a_j0b = [
                accp.tile([128, 1024], BF16, tag=f"a{hl}", name="accj0b")
                for hl in range(2)
            ]
            # pair 1 scores with V st8-15 as fillers
            vg = v_gen(8, 16)
            emit_scores_pair(1, 0, e_j0b, a_j0b, filler=vg)
            for _ in vg:
                pass
            emit_head(2, 0, e_j0b[0], a_j0b[0])
            emit_head(3, 0, e_j0b[1], a_j0b[1])
            emit_gelu(0)

        # ================ phase B: j = 1 + FF + RS ====================
        with (
            tc.tile_pool(name="expj1", bufs=1) as expj1,
            tc.tile_pool(name="xrp", bufs=4) as xrp,
            tc.tile_pool(name="fop", bufs=2) as fop,
        ):
            def ff_chain():
                yield from ff_gen(0, xrp, fop)
                emit_rs(0)
                yield from ff_gen(1, xrp, fop)
                emit_rs(1)

            fch = ff_chain()

            e_j1 = [
                expj1.tile([128, NST * 1024], BF16, tag=f"ej1{hl}", name="ej1")
                for hl in range(2)
            ]
            a_j1 = [
                accp.tile([128, 1024], BF16, tag=f"a{hl}", name="accj1")
                for hl in range(2)
            ]
            emit_scores_pair(0, 1, e_j1, a_j1, filler=fch)
            emit_head(0, 1, e_j1[0], a_j1[0], filler=fch)
            emit_head(1, 1, e_j1[1], a_j1[1], filler=fch)

            e_j1b = [
                expj1.tile([128, NST * 1024], BF16, tag=f"ej1{hl}", name="ej1b")
                for hl in range(2)
            ]
            a_j1b = [
                accp.tile([128, 1024], BF16, tag=f"a{hl}", name="accj1b")
                for hl in range(2)
            ]
            emit_scores_pair(1, 1, e_j1b, a_j1b, filler=fch)
            emit_head(2, 1, e_j1b[0], a_j1b[0], filler=fch)
            emit_head(3, 1, e_j1b[1], a_j1b[1], filler=fch)
            for _ in fch:  # drain any unpulled FF g0/g1 work
                pass
            emit_gelu(1)

            # ---- FF tail: groups 2 and 3 ----
            for _ in ff_gen(2, xrp, fop):
                pass
            emit_rs(2)
            for _ in ff_gen(3, xrp, fop):
                pass
            emit_rs(3)


def make_in_maps(x, Wk, bk, Wv, bv, Wf, bf):
    """Host-side sharding: returns the per-core input dict list."""
    x = np.asarray(x, np.float32)
    Wk = np.asarray(Wk, np.float32)
    Wv = np.asarray(Wv, np.float32)
    Wf = np.asarray(Wf, np.float32)
    bk = np.asarray(bk, np.float32)
    bv = np.asarray(bv, np.float32)
    bf = np.asarray(bf, np.float32)
    mask = np.tril(np.ones((128, 128), np.float32)).T  # mask[k,q]=1 iff k<=q
    in_maps = []
    for c in range(NCORES):
        b, r = c // GROUP, c % GROUP
        xb = x[b]                                    # [S, D]
        xT = np.ascontiguousarray(xb.T).astype(bf16)
        qTs = xT[DKS * r : DKS * (r + 1)]            # heads 4r..4r+3 rows
        bv_s = bv[DVS * r : DVS * (r + 1)]
        in_maps.append({
            "xT": xT,
            "qT": np.ascontiguousarray(qTs),
            "xresq": np.ascontiguousarray((xb + bf[None, :]) * 0.25),
            "wk": np.ascontiguousarray(Wk[:, DKS * r : DKS * (r + 1)]).astype(bf16),
            "wv": np.ascontiguousarray(Wv[:, DVS * r : DVS * (r + 1)]).astype(bf16),
            "wf": np.ascontiguousarray(Wf[DVS * r : DVS * (r + 1), :]).astype(bf16),
            "bkb": bk[None, DKS * r : DKS * (r + 1)].astype(bf16),
            "bvp": np.ascontiguousarray(bv_s.reshape(NM, 128).T).astype(np.float32),
            "maskt": mask.astype(bf16),
        })
    return in_maps


def assemble(results):
    """[8 x [512,1024] bf16] core outputs -> [2,2048,1024] f32."""
    out = np.empty((B, S, D), np.float32)
    for c in range(NCORES):
        b, r = c // GROUP, c % GROUP
        res = np.asarray(results[c]["out"], dtype=np.float32)
        for g in range(4):
            out[b, 512 * g + 128 * r : 512 * g + 128 * r + 128, :] = res[
                128 * g : 128 * (g + 1)
            ]
    return out


def kernel(x, Wk, bk, Wv, bv, Wf, bf, _trace=False, _trace_cores=None):
    global _compiled
    if _compiled is None:
        _compiled = build_program()
    nc = _compiled
    in_maps = make_in_maps(x, Wk, bk, Wv, bv, Wf, bf)
    res = bass_utils.run_bass_kernel_spmd(
        nc,
        in_maps,
        core_ids=list(range(NCORES)),
        trace=_trace,
        trace_cores=_trace_cores,
    )
    out = assemble(res.results)
    kernel.last_result = res
    return out


# revision 18
# speedup vs baseline: 1.2721x; 1.0685x over previous
"""Trainium2 Bass kernel for nn_ExperimentalLayer9 (dense transformer layer).

Layer: x + gelu(attn(x)) @ Wf with
  Q = split_heads(x), K = split_heads(x@Wk+bk), V = split_heads(x@Wv+bv)
  causal softmax (no 1/sqrt(d) scale), exact-erf gelu, residual add.

Sharding over 8 NeuronCores: 2 batch groups x 4-way head/tensor parallel.
Core c handles batch b=c//4 and heads [4r, 4r+4) with r=c%4.

v3: single shared PSUM scope + instruction-level interleaving so the PE
never waits on the scalar-engine exp stream:
 - attention@V in flipped orientation (V stationary, exp-score strips
   moving) -> o^T lands pre-transposed for the FF, no xbar transposes.
 - V projection tiles are pulled as PE fillers inside the j=0 score
   loops (their exp runs concurrently on the scalar engine); FF row
   groups 0/1 are pulled as fillers inside the j=1 attention phase so
   both ReduceScatters overlap compute.
 - softmax denominator: bf16 running k-sums on the DVE, one all-ones
   matmul broadcasts l across partitions, reciprocal_approx_fast (5x
   cheaper than reciprocal), multiply+gelu(+bv as per-partition bias).
 - residual (x+bf)/4 pre-added to every FF partial before the bf16 RS:
   post-RS each core DMAs its shard straight to the bf16 output.
"""

import numpy as np
import ml_dtypes

import concourse.bass as bass
import concourse.mybir as mybir
import concourse.tile as tile
from concourse import bacc
from concourse import bass_utils

# Problem shapes (hardcoded per contest contract).
B, S, D, H, DHID = 2, 2048, 1024, 16, 4096
NCORES = 8
GROUP = 4              # cores per batch group
HPC = 4                # heads per core
DK = 64                # q/k head dim
DV = 256               # v head dim
DKS = HPC * DK         # 256  k-slice per core
DVS = HPC * DV         # 1024 v/hidden slice per core
ROWS = S // GROUP      # 512  output rows per core after ReduceScatter
NM = D // 128          # 8    contraction chunks over d_model
NST = S // 128         # 16   s tiles of 128

BF16 = mybir.dt.bfloat16
F32 = mybir.dt.float32
AF = mybir.ActivationFunctionType
RG = [[0, 1, 2, 3], [4, 5, 6, 7]]

bf16 = ml_dtypes.bfloat16

_compiled = None


def build_program():
    nc = bacc.Bacc(
        "TRN2",
        target_bir_lowering=False,
        debug=False,
        enable_asserts=True,
        num_devices=NCORES,
    )

    xT = nc.dram_tensor("xT", [D, S], BF16, kind="ExternalInput").ap()
    xresq = nc.dram_tensor("xresq", [S, D], BF16, kind="ExternalInput").ap()
    wk = nc.dram_tensor("wk", [D, DKS], BF16, kind="ExternalInput").ap()
    wv = nc.dram_tensor("wv", [D, DVS], BF16, kind="ExternalInput").ap()
    wf = nc.dram_tensor("wf", [DVS, D], BF16, kind="ExternalInput").ap()
    bkb = nc.dram_tensor("bkb", [1, DKS], BF16, kind="ExternalInput").ap()
    bvp = nc.dram_tensor("bvp", [128, NM], F32, kind="ExternalInput").ap()
    maskt = nc.dram_tensor("maskt", [128, 128], BF16, kind="ExternalInput").ap()
    out = nc.dram_tensor("out", [ROWS, D], BF16, kind="ExternalOutput").ap()

    with tile.TileContext(nc) as tc:
        _body(nc, tc, xT, xresq, wk, wv, wf, bkb, bvp, maskt, out)

    nc.compile()
    return nc


def _body(nc, tc, xT, xresq, wk, wv, wf, bkb, bvp, maskt, out):
    with (
        tc.tile_pool(name="const", bufs=1) as constp,
        tc.tile_pool(name="kv", bufs=1) as kvp,
        tc.tile_pool(name="gotp", bufs=1) as gotp,
        tc.tile_pool(name="wfp", bufs=1) as wfp,
        tc.tile_pool(name="accp", bufs=2) as accp,
        tc.tile_pool(name="rcpp", bufs=1) as rcpp,
        tc.tile_pool(name="dram", bufs=1, space="DRAM") as dramp,
        tc.tile_pool(name="psS", bufs=2, space="PSUM") as psS,
        tc.tile_pool(name="psV", bufs=1, space="PSUM") as psV,
        tc.tile_pool(name="psF", bufs=2, space="PSUM") as psF,
    ):
        # ---- constants ------------------------------------------------
        ones_sb = constp.tile([1, 512], BF16)
        nc.vector.memset(ones_sb[:], 1.0)
        allones = constp.tile([128, 128], BF16)
        nc.vector.memset(allones[:], 1.0)
        mask_sb = constp.tile([128, 128], BF16)
        nc.scalar.dma_start(mask_sb[:], maskt[:])
        bk_sb = constp.tile([1, DKS], BF16)
        nc.scalar.dma_start(bk_sb[:], bkb[:])
        bv_sb = constp.tile([128, NM], F32)
        nc.scalar.dma_start(bv_sb[:], bvp[:])

        # ---- persistent SBUF + loads ----------------------------------
        # q rows are xT rows 0..255 (host rolls the d axis per core)
        qT_sb = kvp.tile([128, 2 * S], BF16)
        kt_sb = kvp.tile([128, 2 * S], BF16)   # K^T rows dk%128, chunk dk//128
        # V: col = kt*1024 + h*256 + dv   (per 128-row k tile)
        v_sb = kvp.tile([128, NST * DVS], BF16)
        got_sb = gotp.tile([128, NM * S], BF16)  # gelu(o)^T, hc-major x q
        wf_sb = wfp.tile([128, NM * D], BF16)

        # gpsimd queue: wk (needed first), then collective warmup, then wf
        wk_sb = kvp.tile([128, NM * DKS], BF16)
        for m in range(NM):
            nc.gpsimd.dma_start(
                wk_sb[:, m * DKS : (m + 1) * DKS], wk[m * 128 : (m + 1) * 128, :]
            )
        warm_in = dramp.tile([4, 16], BF16, tag="warm_in")
        warm_out = dramp.tile([1, 16], BF16, tag="warm_out")
        nc.gpsimd.dma_start(
            warm_in[:].rearrange("a b -> (a b)")[None, :], ones_sb[0:1, 0:64]
        )
        nc.gpsimd.collective_compute(
            "ReduceScatter",
            mybir.AluOpType.add,
            replica_groups=RG,
            ins=[warm_in.opt()],
            outs=[warm_out.opt()],
        )
        for m in range(NM):
            nc.gpsimd.dma_start(
                wf_sb[:, m * D : (m + 1) * D], wf[m * 128 : (m + 1) * 128, :]
            )

        partials = [
            dramp.tile([512, D], BF16, tag=f"part{g}", name=f"part{g}")
            for g in range(4)
        ]
        rss = [
            dramp.tile([128, D], BF16, tag=f"rs{g}", name=f"rs{g}")
            for g in range(4)
        ]

        # ---- emission helpers -----------------------------------------
        def emit_scores_pair(pair, j, exps2, acc2, filler=None, rate=1):
            """Causal scores^T -> exp (no max-sub) -> mask -> bf16 running
            k-sums.  Two heads run row-tiled on the 64-row PE halves.
            One filler unit is pulled per (kt, hl) to keep the PE fed
            while the scalar engine streams exp."""
            nkt = 8 * j + 8
            co = pair * S
            for kt in range(nkt):
                t = kt - 8 * j
                toff = max(t, 0) * 128
                for hl in range(2):
                    po = 64 * hl
                    for qh in range(2):
                        a = max(qh * 512, toff)
                        b_ = qh * 512 + 512
                        if a >= b_:
                            continue
                        ps = psS.tile([128, 512], F32, tag="st", name="st")
                        nc.tensor.matmul(
                            ps[:, a - qh * 512 : 512],
                            kt_sb[po : po + 64, co + kt * 128 : co + kt * 128 + 128],
                            qT_sb[po : po + 64, co + j * 1024 + a : co + j * 1024 + b_],
                            start=True,
                            stop=True,
                            tile_position=(po, 0),
                        )
                        nc.scalar.activation(
                            exps2[hl][:, kt * 1024 + a : kt * 1024 + b_],
                            ps[:, a - qh * 512 : 512],
                            AF.Exp,
                        )
                    if t >= 0:  # mask the diagonal 128x128 block
                        blk = exps2[hl][:, kt * 1024 + toff : kt * 1024 + toff + 128]
                        nc.vector.tensor_mul(blk, blk, mask_sb[:])
                    # bf16 running sum over k tiles (softmax denominator)
                    if kt == 0:
                        nc.vector.tensor_copy(acc2[hl][:], exps2[hl][:, 0:1024])
                    else:
                        nc.vector.tensor_add(
                            acc2[hl][:, toff:1024],
                            acc2[hl][:, toff:1024],
                            exps2[hl][:, kt * 1024 + toff : (kt + 1) * 1024],
                        )
                    if filler is not None:
                        for _ in range(rate):
                            next(filler, None)

        def emit_head(h, j, exps, acc, filler=None):
            """Flipped AV (V stationary) + l broadcast + normalize.
            l-matmul + fast reciprocal are emitted between the two dv
            chunks so the reciprocal overlaps chunk-1 matmuls and the
            psV slot for the next head frees immediately."""
            nkt = 8 * j + 8
            psvs = []
            rcp = rcpp.tile([128, 1024], F32, tag=f"r{h % 2}", name="rcp")
            for c in range(2):
                psv = psV.tile([128, 1024], F32, tag=f"av{c}", name="psv")
                for kt in range(nkt):
                    toff = max(kt - 8 * j, 0) * 128
                    vcol = kt * DVS + h * 256 + c * 128
                    for qh in range(2):
                        a = max(qh * 512, toff)
                        b_ = qh * 512 + 512
                        if a >= b_:
                            continue
                        nc.tensor.matmul(
                            psv[:, a:b_],
                            v_sb[:, vcol : vcol + 128],
                            exps[:, kt * 1024 + a : kt * 1024 + b_],
                            start=(kt == 0),
                            stop=(kt == nkt - 1),
                            skip_group_check=True,
                        )
                    if c == 0 and filler is not None:
                        next(filler, None)
                psvs.append(psv)
                if c == 0:
                    # l(q) replicated across partitions: allones.T @ acc
                    for half in range(2):
                        rb = psS.tile([128, 512], F32, tag="st", name="rb")
                        nc.tensor.matmul(
                            rb[:], allones[:], acc[:, half * 512 : half * 512 + 512],
                            start=True, stop=True,
                        )
                        nc.vector.reciprocal_approx_fast(
                            rcp[:, half * 512 : half * 512 + 512], rb[:]
                        )
            for c in range(2):
                hc = 2 * h + c
                gs = got_sb[:, hc * S + j * 1024 : hc * S + j * 1024 + 1024]
                nc.vector.tensor_mul(gs, psvs[c][:], rcp[:])
                nc.vector.tensor_scalar_add(gs, gs, bv_sb[:, hc : hc + 1])

        def emit_gelu(j):
            for hc in range(NM):
                gs = got_sb[:, hc * S + j * 1024 : hc * S + j * 1024 + 1024]
                nc.scalar.activation(gs, gs, AF.Gelu)

        def ff_gen(g, xrp, fop):
            """FF partial for row group g; one yield per 2 matmuls."""
            for cc in range(4):
                q0 = g * 512 + cc * 128
                xr = xrp.tile([128, D], BF16, tag="xr", name="xr")
                nc.sync.dma_start(xr[:], xresq[q0 : q0 + 128, :])
                ps0 = psF.tile([128, 512], F32, tag="ff", name="ps0")
                ps1 = psF.tile([128, 512], F32, tag="ff", name="ps1")
                for hc in range(NM):
                    lhsT = got_sb[:, hc * S + q0 : hc * S + q0 + 128]
                    nc.tensor.matmul(
                        ps0[:], lhsT, wf_sb[:, hc * D : hc * D + 512],
                        start=(hc == 0), stop=(hc == NM - 1),
                    )
                    nc.tensor.matmul(
                        ps1[:], lhsT, wf_sb[:, hc * D + 512 : hc * D + 1024],
                        start=(hc == 0), stop=(hc == NM - 1),
                    )
                    yield
                # pre-add (x+bf)/4 so the RS carries the residual
                fo = fop.tile([128, D], BF16, tag="fo", name="fo")
                nc.vector.tensor_add(fo[:, 0:512], ps0[:], xr[:, 0:512])
                nc.vector.tensor_add(fo[:, 512:1024], ps1[:], xr[:, 512:1024])
                nc.sync.dma_start(partials[g][cc * 128 : (cc + 1) * 128, :], fo[:])

        def emit_rs(g):
            nc.gpsimd.collective_compute(
                "ReduceScatter",
                mybir.AluOpType.add,
                replica_groups=RG,
                ins=[partials[g].opt()],
                outs=[rss[g].opt()],
            )
            nc.gpsimd.dma_start(out[g * 128 : (g + 1) * 128, :], rss[g][:])

        # ================ phase A: projections + j = 0 ================
        with (
            tc.tile_pool(name="xtp", bufs=1) as xtp,
            tc.tile_pool(name="expj0", bufs=1) as expj0,
        ):
            xT_sb = xtp.tile([128, NM * S], BF16)
            for m in range(NM):  # split xT across sync+scalar DMA channels
                q = nc.sync if m < 4 else nc.scalar
                q.dma_start(
                    xT_sb[:, m * S : (m + 1) * S], xT[m * 128 : (m + 1) * 128, :]
                )
            for m in range(2):
                nc.sync.dma_start(
                    qT_sb[:, m * S : (m + 1) * S], xT[m * 128 : (m + 1) * 128, :]
                )
            wv_sb = xtp.tile([128, NM * DVS], BF16)
            for m in range(NM):
                nc.scalar.dma_start(
                    wv_sb[:, m * DVS : (m + 1) * DVS], wv[m * 128 : (m + 1) * 128, :]
                )

            # K^T proj: 2 passes of 4 st-tiles, m-outer so the PE starts
            # as soon as xT chunk 0 lands (bias seeds run immediately).
            for dkt in range(2):
                ktile = []
                for st in range(4):
                    pool = psS if st < 2 else psF
                    kps = pool.tile([128, 512], F32, tag="st" if st < 2 else "ff", name="kps")
                    nc.tensor.matmul(
                        kps[:],
                        bk_sb[:, dkt * 128 : (dkt + 1) * 128],
                        ones_sb[:, 0:512],
                        start=True,
                        stop=False,
                    )
                    ktile.append(kps)
                for m in range(NM):
                    for st in range(4):
                        nc.tensor.matmul(
                            ktile[st][:],
                            wk_sb[:, m * DKS + dkt * 128 : m * DKS + dkt * 128 + 128],
                            xT_sb[:, m * S + st * 512 : m * S + st * 512 + 512],
                            start=False,
                            stop=(m == NM - 1),
                        )
                for st in range(4):
                    nc.vector.tensor_copy(
                        kt_sb[:, dkt * S + st * 512 : dkt * S + st * 512 + 512],
                        ktile[st][:],
                    )

            def v_gen(st0, st1):
                """V proj units (st, dvh): 8 matmuls + a DVE copy each."""
                for st in range(st0, st1):
                    for dvh in range(2):
                        vps = psV.tile(
                            [128, 512], F32, tag=f"av{dvh}", name="vps"
                        )
                        for m in range(NM):
                            nc.tensor.matmul(
                                vps[:],
                                xT_sb[:, m * S + st * 128 : m * S + st * 128 + 128],
                                wv_sb[:, m * DVS + dvh * 512 : m * DVS + dvh * 512 + 512],
                                start=(m == 0),
                                stop=(m == NM - 1),
                            )
                        nc.vector.tensor_copy(
                            v_sb[:, st * DVS + dvh * 512 : st * DVS + dvh * 512 + 512],
                            vps[:],
                        )
                        yield

            e_j0 = [
                expj0.tile([128, 8 * 1024], BF16, tag=f"ej0{hl}", name="ej0")
                for hl in range(2)
            ]
            a_j0 = [
                accp.tile([128, 1024], BF16, tag=f"a{hl}", name="accj0")
                for hl in range(2)
            ]
            # pair 0 scores with V st0-7 as fillers (16 pulls, 16 units)
            vg = v_gen(0, 8)
            emit_scores_pair(0, 0, e_j0, a_j0, filler=vg)
            for _ in vg:
                pass
            emit_head(0, 0, e_j0[0], a_j0[0])
            emit_head(1, 0, e_j0[1], a_j0[1])
            e_j0b = [
                expj0.tile([128, 8 * 1024], BF16, tag=f"ej0{hl}", name="ej0b")
                for hl in range(2)
            ]
            a_j0b = [
                accp.tile([128, 1024], BF16, tag=f"a{hl}", name="accj0b")
                for hl in range(2)
            ]
            # pair 1 scores with V st8-15 as fillers
            vg = v_gen(8, 16)
            emit_scores_pair(1, 0, e_j0b, a_j0b, filler=vg)
            for _ in vg:
                pass
            emit_head(2, 0, e_j0b[0], a_j0b[0])
            emit_head(3, 0, e_j0b[1], a_j0b[1])
            emit_gelu(0)

        # ================ phase B: j = 1 + FF + RS ====================
        with (
            tc.tile_pool(name="expj1", bufs=1) as expj1,
            tc.tile_pool(name="xrp", bufs=4) as xrp,
            tc.tile_pool(name="fop", bufs=2) as fop,
        ):
            def ff_chain():
                yield from ff_gen(0, xrp, fop)
                emit_rs(0)
                yield from ff_gen(1, xrp, fop)
                emit_rs(1)

            fch = ff_chain()

            e_j1 = [
                expj1.tile([128, NST * 1024], BF16, tag=f"ej1{hl}", name="ej1")
                for hl in range(2)
            ]
            a_j1 = [
                accp.tile([128, 1024], BF16, tag=f"a{hl}", name="accj1")
                for hl in range(2)
            ]
            emit_scores_pair(0, 1, e_j1, a_j1, filler=fch, rate=2)
            emit_head(0, 1, e_j1[0], a_j1[0], filler=fch)
            emit_head(1, 1, e_j1[1], a_j1[1], filler=fch)

            e_j1b = [
                expj1.tile([128, NST * 1024], BF16, tag=f"ej1{hl}", name="ej1b")
                for hl in range(2)
            ]
            a_j1b = [
                accp.tile([128, 1024], BF16, tag=f"a{hl}", name="accj1b")
                for hl in range(2)
            ]
            emit_scores_pair(1, 1, e_j1b, a_j1b, filler=fch)
            emit_head(2, 1, e_j1b[0], a_j1b[0], filler=fch)
            emit_head(3, 1, e_j1b[1], a_j1b[1], filler=fch)
            for _ in fch:  # drain any unpulled FF g0/g1 work
                pass
            emit_gelu(1)

            # ---- FF tail: groups 2 and 3 ----
            for _ in ff_gen(2, xrp, fop):
                pass
            emit_rs(2)
            for _ in ff_gen(3, xrp, fop):
                pass
            emit_rs(3)


def make_in_maps(x, Wk, bk, Wv, bv, Wf, bf):
    """Host-side sharding: returns the per-core input dict list."""
    x = np.asarray(x, np.float32)
    Wk = np.asarray(Wk, np.float32)
    Wv = np.asarray(Wv, np.float32)
    Wf = np.asarray(Wf, np.float32)
    bk = np.asarray(bk, np.float32)
    bv = np.asarray(bv, np.float32)
    bf = np.asarray(bf, np.float32)
    mask = np.tril(np.ones((128, 128), np.float32)).T  # mask[k,q]=1 iff k<=q
    in_maps = []
    for c in range(NCORES):
        b, r = c // GROUP, c % GROUP
        xb = x[b]                                    # [S, D]
        # Roll the d axis so this core's q-head rows sit at xT rows 0..255
        # (jointly rolling xT / Wk / Wv rows leaves the contraction
        # invariant and keeps the device program SPMD-identical).
        sh = -DKS * r
        xTr = np.roll(xb.T, sh, axis=0)
        bv_s = bv[DVS * r : DVS * (r + 1)]
        in_maps.append({
            "xT": np.ascontiguousarray(xTr).astype(bf16),
            "xresq": np.ascontiguousarray((xb + bf[None, :]) * 0.25).astype(bf16),
            "wk": np.ascontiguousarray(
                np.roll(Wk[:, DKS * r : DKS * (r + 1)], sh, axis=0)
            ).astype(bf16),
            "wv": np.ascontiguousarray(
                np.roll(Wv[:, DVS * r : DVS * (r + 1)], sh, axis=0)
            ).astype(bf16),
            "wf": np.ascontiguousarray(Wf[DVS * r : DVS * (r + 1), :]).astype(bf16),
            "bkb": bk[None, DKS * r : DKS * (r + 1)].astype(bf16),
            "bvp": np.ascontiguousarray(bv_s.reshape(NM, 128).T).astype(np.float32),
            "maskt": mask.astype(bf16),
        })
    return in_maps


def assemble(results):
    """[8 x [512,1024] bf16] core outputs -> [2,2048,1024] f32."""
    out = np.empty((B, S, D), np.float32)
    for c in range(NCORES):
        b, r = c // GROUP, c % GROUP
        res = np.asarray(results[c]["out"], dtype=np.float32)
        for g in range(4):
            out[b, 512 * g + 128 * r : 512 * g + 128 * r + 128, :] = res[
                128 * g : 128 * (g + 1)
            ]
    return out


def kernel(x, Wk, bk, Wv, bv, Wf, bf, _trace=False, _trace_cores=None):
    global _compiled
    if _compiled is None:
        _compiled = build_program()
    nc = _compiled
    in_maps = make_in_maps(x, Wk, bk, Wv, bv, Wf, bf)
    res = bass_utils.run_bass_kernel_spmd(
        nc,
        in_maps,
        core_ids=list(range(NCORES)),
        trace=_trace,
        trace_cores=_trace_cores,
    )
    out = assemble(res.results)
    kernel.last_result = res
    return out
